# revision 1
# baseline (speedup 1.0000x reference)
"""Trainium2 Bass kernel for a gaussian-moment energy model (GNN message passing).

Strategy (8 NeuronCores, SPMD single program):
  - Host: partition atoms into 8 contiguous shards of 12500 (padded to 12544
    = 98 blocks of 128). Edges owned by the receiver atom i. Sort edges by i,
    group into 32-atom windows, pad each window's edge list to a multiple of
    128 (KT tiles of 128 edges per window, KT = global max). Pre-gather
    R[i]/R[j] per edge slot (the "halo gather" of the sharding hint) and the
    per-edge one-hot window-assignment matrix T (exact 0/1 in bf16).
  - Device per core: edge pipeline (dr, r, unit dir, 16 gaussian basis
    functions, b*d features) -> one-hot scatter matmuls into PSUM (per-atom
    M0/M1 moments) -> gaussian-moment contraction G1 (symmetry-reduced) ->
    PE transpose -> 3-layer silu MLP -> per-atom scale/shift -> scalar sum.
  - Host: sum the 8 per-core partial energies.
"""

import math
import numpy as np
import ml_dtypes

# ---------------------------------------------------------------- constants
N_ATOMS = 100_000
N_EDGES = 1_600_000
N_RADIAL = 16
HIDDEN = 512
EPS = 1e-8

N_CORES = 8
APC = 12_500          # real atoms per core
APC_PAD = 12_544      # 98 blocks of 128
WIN = 32              # atoms per scatter window (psum col-group)
NWIN = APC_PAD // WIN  # 392
NBLK = APC_PAD // 128  # 98
NPAIR = 144           # 16 k1 x 9 j symmetric-pair slots
NFEAT = 16 + NPAIR    # 160
ROW_CH = 512          # atoms per MLP chunk
N_CH = APC_PAD // ROW_CH if APC_PAD % ROW_CH == 0 else APC_PAD // ROW_CH + 1  # 25
EMAT_P = N_CH         # 25 partitions for the readout matrix


def _preprocess(R, Z, idx, centers, width, W1, b1, W2, b2, W3, b3, scale, shift,
                box=None, offsets=None):
    """Host-side graph partitioning / sharding. Index manipulation + gathers
    only (plus compile-time constant folding of the MLP weights)."""
    R = np.asarray(R, np.float32)
    Z = np.asarray(Z)
    idx_i = np.asarray(idx[0]).astype(np.int64)
    idx_j = np.asarray(idx[1]).astype(np.int64)
    centers = np.asarray(centers, np.float32)
    width = float(np.asarray(width))

    order = np.argsort(idx_i, kind="stable")
    si = idx_i[order]
    sj = idx_j[order]

    core = si // APC                      # owner core per edge
    la = si - core * APC                  # local atom id [0, APC)
    gw = core * NWIN + la // WIN          # global window id

    counts = np.bincount(gw, minlength=N_CORES * NWIN)
    kt = max(1, int(math.ceil(counts.max() / 128.0)))
    tt = NWIN * kt                        # tiles per core
    nslot = tt * 128                      # edge slots per core

    starts = np.zeros(N_CORES * NWIN, np.int64)
    np.cumsum(counts[:-1], out=starts[1:])
    rank = np.arange(si.shape[0], dtype=np.int64) - starts[gw]
    wloc = gw % NWIN
    slot = wloc * (kt * 128) + rank       # slot within core

    # per-edge-slot positions; pads get r ~ 1e6 so every basis fn underflows to 0
    epos = np.zeros((N_CORES, nslot, 6), np.float32)
    epos[:, :, 3] = 1.0e6
    epos[core, slot, 0:3] = R[si]
    epos[core, slot, 3:6] = R[sj]

    tmat = np.zeros((N_CORES, nslot, 32), ml_dtypes.bfloat16)
    tmat[core, slot, la % WIN] = 1.0

    # DRAM layouts: partition dim = edge slot within tile (p = slot % 128)
    epos = np.ascontiguousarray(
        epos.reshape(N_CORES, tt, 128, 6).transpose(0, 2, 1, 3).reshape(N_CORES, 128, tt * 6)
    )
    tmat = np.ascontiguousarray(
        tmat.reshape(N_CORES, tt, 128, 32).transpose(0, 2, 1, 3).reshape(N_CORES, 128, tt * 32)
    )

    # --- weight constant-folding (symmetry reduction of the G1 contraction) ---
    W1 = np.asarray(W1, np.float32)
    w1g = W1[16:].reshape(16, 16, HIDDEN)
    w1s = np.zeros((NFEAT, HIDDEN), np.float32)
    w1s[0:16] = W1[0:16]
    for k1 in range(16):
        for j in range(9):
            k2 = (k1 + j) % 16
            r = 16 + k1 * 9 + j
            if j == 0:
                w1s[r] = w1g[k1, k1]
            elif j == 8:
                w1s[r] = 0.5 * (w1g[k1, k2] + w1g[k2, k1])
            else:
                w1s[r] = w1g[k1, k2] + w1g[k2, k1]

    W2 = np.asarray(W2, np.float32)
    W3 = np.asarray(W3, np.float32).reshape(HIDDEN)
    w1a = w1s[0:128].copy()                                           # [128, 512]
    w1b = w1s[128:160].copy()                                         # [32, 512]
    w2r = np.ascontiguousarray(
        W2.reshape(4, 128, HIDDEN).transpose(1, 0, 2).reshape(128, 4 * HIDDEN)
    )                                                                 # [128, 2048]
    w3r = np.ascontiguousarray(W3.reshape(4, 128).T)                  # [128, 4]
    b1t = np.ascontiguousarray(np.asarray(b1, np.float32).reshape(4, 128).T)     # [128, 4]
    b2t = np.ascontiguousarray(np.asarray(b2, np.float32).reshape(4, 128).T)
    b3s = np.asarray(b3, np.float32).reshape(1, 1)

    # per-atom scale / shift, [25, 512] per core, zero on pad atoms
    scale = np.asarray(scale, np.float32)
    shift = np.asarray(shift, np.float32)
    smat = np.zeros((N_CORES, N_CH * ROW_CH), np.float32)
    shmat = np.zeros((N_CORES, N_CH * ROW_CH), np.float32)
    for c in range(N_CORES):
        zc = Z[c * APC:(c + 1) * APC]
        smat[c, :APC] = scale[zc]
        shmat[c, :APC] = shift[zc]
    smat = smat.reshape(N_CORES, N_CH, ROW_CH)
    shmat = shmat.reshape(N_CORES, N_CH, ROW_CH)

    consts = dict(
        kt=kt, tt=tt,
        neg_beta=-1.0 / (2.0 * width * width),
        centers=[float(c) for c in centers],
    )
    per_core = dict(epos=epos, tmat=tmat, smat=smat, shmat=shmat)
    shared = dict(w1a=w1a, w1b=w1b, w2r=w2r, w3r=w3r, b1t=b1t, b2t=b2t, b3s=b3s)
    return consts, per_core, shared


# ------------------------------------------------------- numpy mirror (test)
def _mirror_core(c, consts, per_core, shared):
    """Pure-numpy emulation of the planned device program for one core, with
    the same dtype casts at the same places. Used by test.py for validation."""
    kt, tt = consts["kt"], consts["tt"]
    bf16 = ml_dtypes.bfloat16
    epos = per_core["epos"][c].reshape(128, tt, 6).transpose(1, 0, 2)  # [tt,128,6]
    tmat = per_core["tmat"][c].reshape(128, tt, 32).transpose(1, 0, 2)  # [tt,128,32]

    ri = epos[:, :, 0:3]
    rj = epos[:, :, 3:6]
    dr = rj - ri                                       # fp32
    r2 = (dr * dr).sum(-1)
    r = np.sqrt(r2)
    rinv = (1.0 / (r + EPS)).astype(np.float32)
    d = (dr * rinv[:, :, None]).astype(np.float32)
    dexp = np.repeat(d, 16, axis=-1).astype(bf16)      # [tt,128,48]
    bsq = (r[:, :, None] - np.asarray(consts["centers"], np.float32)[None, None, :]) ** 2
    b = np.exp(consts["neg_beta"] * bsq).astype(bf16)  # [tt,128,16]
    bd = (np.repeat(b, 3, axis=0).reshape(tt, 3, 128, 16).transpose(0, 2, 1, 3).reshape(tt, 128, 48).astype(np.float32)
          * dexp.astype(np.float32)).astype(bf16)
    feat_e = np.concatenate([b, bd], axis=-1)          # [tt,128,64] bf16

    # scatter: psum accumulate per window in fp32
    mfeat = np.zeros((APC_PAD, 64), np.float32)
    for w in range(NWIN):
        acc = np.zeros((32, 64), np.float32)
        for t in range(w * kt, (w + 1) * kt):
            acc += tmat[t].astype(np.float32).T @ feat_e[t].astype(np.float32)
        mfeat[w * 32:(w + 1) * 32] = acc

    m0 = mfeat[:, 0:16]
    m1 = mfeat[:, 16:64]                                # [(A),48] (d,k) d-major
    k1 = np.arange(16)[:, None, None]
    j = np.arange(9)[None, :, None]
    d = np.arange(3)[None, None, :]
    i0 = d * 16 + k1 + 0 * j                            # [16,9,3]
    i1 = d * 16 + (k1 + j) % 16
    a = m1[:, i0.reshape(-1)].reshape(-1, 16, 9, 3).astype(np.float32)
    bb = m1[:, i1.reshape(-1)].reshape(-1, 16, 9, 3).astype(np.float32)
    prods = (a * bb)  # [A,16,9,3]
    g1 = prods.sum(-1)
    feat = np.concatenate([m0, g1.reshape(-1, NPAIR)], axis=-1)  # [A,160]

    # MLP modeled as fp32 (device uses float32r matmuls, ~2^-13 per-element)
    w1 = np.concatenate([shared["w1a"], shared["w1b"]], 0).astype(np.float32)
    b1 = shared["b1t"].T.reshape(HIDDEN)
    b2 = shared["b2t"].T.reshape(HIDDEN)
    w2 = shared["w2r"].reshape(128, 4, HIDDEN).transpose(1, 0, 2).reshape(HIDDEN, HIDDEN).astype(np.float32)
    w3 = shared["w3r"].T.reshape(HIDDEN).astype(np.float32)

    h = feat.astype(np.float32) @ w1 + b1
    h = h / (1 + np.exp(-h))
    h = h @ w2 + b2
    h = h / (1 + np.exp(-h))
    h3 = h @ w3 + float(shared["b3s"][0, 0])
    sm = per_core["smat"][c].reshape(-1)[:APC_PAD]
    sh = per_core["shmat"][c].reshape(-1)[:APC_PAD]
    return float(np.sum(sm * h3 + sh, dtype=np.float64))


def mirror(inputs):
    consts, per_core, shared = _preprocess(**inputs)
    return np.float32(sum(_mirror_core(c, consts, per_core, shared) for c in range(N_CORES)))


# ================================================================ device code
def _split_multi_waits(nc, mybir, max_waits=1):
    """Current walrus codegen rejects instructions carrying more than one
    semaphore wait. Hoist surplus waits onto NOPs inserted just before the
    instruction on the same (in-order) engine queue."""
    for f in nc.m.functions:
        for bb in f.blocks:
            out, changed = [], False
            for inst in bb.instructions:
                si = inst.sync_info
                waits = list(si.on_wait) if (si and si.on_wait) else []
                if len(waits) > max_waits:
                    extra, si.on_wait = waits[:-max_waits], waits[-max_waits:]
                    for k, w in enumerate(extra):
                        nop = mybir.InstNoOp(name=f"{inst.name}-wsplit{k}", ins=[], outs=[])
                        nop.engine = inst.engine
                        nop.sync_info = mybir.SyncInfo(on_wait=[w], on_update=[])
                        out.append(nop)
                    changed = True
                out.append(inst)
            if changed:
                bb.instructions = out


_PROG_CACHE = {}


def _get_program(kt, centers, neg_beta, b3val, num_devices=N_CORES, fix_waits=True):
    import concourse.bass as bass
    import concourse.mybir as mybir
    import concourse.tile as tile
    from concourse.tile import ScopedClock
    from concourse.masks import make_identity

    key = (kt, tuple(centers), neg_beta, b3val, num_devices, fix_waits)
    if key in _PROG_CACHE:
        return _PROG_CACHE[key]

    class SplitDrainTileContext(tile.TileContext):
        def _drain_and_barrier(self, tick_clock, wait_clock):
            drain_inst = self.nc.sync.drain()
            wait_clock.add_sem_waits(
                drain_inst.ins, ScopedClock({None: tick_clock.global_clock})
            )
            si = drain_inst.ins.sync_info
            waits = list(si.on_wait or [])
            if len(waits) > 1:
                si.on_wait = waits[:1]
                for w in waits[1:]:
                    extra = self.nc.sync.drain()
                    extra.ins.sync_info = mybir.SyncInfo(on_wait=[w], on_update=[])
            self.nc.all_engine_barrier()
            assert self.sems is not None
            popped = self.nc._tile_sem_poison_stack.pop()
            assert popped is self._sem_poison
            self.nc.clear_and_free_semaphores(list(self.sems.allocated().values()))
            self.nc.all_engine_barrier()
            _split_multi_waits(self.nc, mybir)

    TC = SplitDrainTileContext if fix_waits else tile.TileContext

    f32 = mybir.dt.float32
    f32r = mybir.dt.float32r
    bf = mybir.dt.bfloat16
    tt = NWIN * kt
    SC_BLK = 4                       # atom blocks per superchunk
    sc_blocks = [SC_BLK] * (NBLK // SC_BLK) + ([NBLK % SC_BLK] if NBLK % SC_BLK else [])

    nc = bass.Bass("TRN2", target_bir_lowering=False, debug=False, num_devices=num_devices)
    epos_d = nc.dram_tensor("epos", [128, tt * 6], f32, kind="ExternalInput")
    tmat_d = nc.dram_tensor("tmat", [128, tt * 32], bf, kind="ExternalInput")
    w1a_d = nc.dram_tensor("w1a", [128, HIDDEN], f32r, kind="ExternalInput")
    w1b_d = nc.dram_tensor("w1b", [32, HIDDEN], f32r, kind="ExternalInput")
    w2r_d = nc.dram_tensor("w2r", [128, 4 * HIDDEN], f32r, kind="ExternalInput")
    w3r_d = nc.dram_tensor("w3r", [128, 4], f32r, kind="ExternalInput")
    b1t_d = nc.dram_tensor("b1t", [128, 4], f32, kind="ExternalInput")
    b2t_d = nc.dram_tensor("b2t", [128, 4], f32, kind="ExternalInput")
    b3s_d = nc.dram_tensor("b3s", [1, 1], f32, kind="ExternalInput")
    smat_d = nc.dram_tensor("smat", [EMAT_P, ROW_CH], f32, kind="ExternalInput")
    shm_d = nc.dram_tensor("shm", [EMAT_P, ROW_CH], f32, kind="ExternalInput")
    ft0_d = nc.dram_tensor("ft0", [128, APC_PAD], f32r)
    ft1_d = nc.dram_tensor("ft1", [32, APC_PAD], f32r)
    eout_d = nc.dram_tensor("eout", [1, 1], f32, kind="ExternalOutput")

    with TC(nc) as tc:
        with tc.tile_pool(name="const", bufs=1) as cpool:
            ident = cpool.tile([128, 128], f32)
            make_identity(nc, ident[:])
            ident_r = cpool.tile([128, 128], f32r)
            nc.vector.tensor_copy(out=ident_r[:], in_=ident[:])
            identr = ident_r[:]
            negc = cpool.tile([128, 16], f32)
            for k in range(16):
                nc.vector.memset(negc[:, k:k + 1], float(-centers[k]))
            w1a_t = cpool.tile([128, HIDDEN], f32r)
            w1b_t = cpool.tile([32, HIDDEN], f32r)
            w2r_t = cpool.tile([128, 4 * HIDDEN], f32r)
            w3r_t = cpool.tile([128, 4], f32r)
            b1t_t = cpool.tile([128, 4], f32)
            b2t_t = cpool.tile([128, 4], f32)
            b3s_t = cpool.tile([1, 1], f32)
            smat_t = cpool.tile([EMAT_P, ROW_CH], f32)
            shm_t = cpool.tile([EMAT_P, ROW_CH], f32)
            emat_t = cpool.tile([EMAT_P, ROW_CH], f32)
            for dst, src in [(w1a_t, w1a_d), (w1b_t, w1b_d), (w2r_t, w2r_d),
                             (w3r_t, w3r_d), (b1t_t, b1t_d), (b2t_t, b2t_d),
                             (b3s_t, b3s_d), (smat_t, smat_d), (shm_t, shm_d)]:
                nc.sync.dma_start(out=dst[:], in_=src.ap())
            nc.vector.memset(emat_t[:], 0.0)

            # ---------------- Phase A: edges -> transposed per-atom features
            with tc.tile_pool(name="eio", bufs=2) as eio, \
                 tc.tile_pool(name="emid", bufs=2) as emid, \
                 tc.tile_pool(name="msc", bufs=2) as msc, \
                 tc.tile_pool(name="psA", bufs=3, space="PSUM") as psA_p, \
                 tc.tile_pool(name="psT", bufs=2, space="PSUM") as psT_p:
                blk0 = 0
                for nblk in sc_blocks:
                    nw = nblk * 4                  # windows in this superchunk
                    nt = nw * kt                   # edge tiles
                    t0 = blk0 * 4 * kt
                    a0 = blk0 * 128

                    epos_t = eio.tile([128, nt, 6], f32, tag="epos")
                    tmat_t = eio.tile([128, nt, 32], bf, tag="tmat")
                    nc.sync.dma_start(out=epos_t[:].rearrange("p a b -> p (a b)"),
                                      in_=epos_d.ap()[:, t0 * 6:(t0 + nt) * 6])
                    nc.sync.dma_start(out=tmat_t[:].rearrange("p a b -> p (a b)"),
                                      in_=tmat_d.ap()[:, t0 * 32:(t0 + nt) * 32])

                    drt = emid.tile([128, nt, 3], f32, tag="drt")
                    sqt = emid.tile([128, nt, 3], f32, tag="sqt")
                    r2t = emid.tile([128, nt], f32, tag="r2t")
                    rt = emid.tile([128, nt], f32, tag="rt")
                    rit = emid.tile([128, nt], f32, tag="rit")
                    dt_ = emid.tile([128, nt, 3], f32, tag="dt")
                    dexp = emid.tile([128, nt, 3, 16], bf, tag="dexp")
                    bsqt = emid.tile([128, nt, 16], f32, tag="bsqt")
                    feat_e = emid.tile([128, nt, 64], bf, tag="feate")

                    nc.vector.tensor_tensor(out=drt[:], in0=epos_t[:, :, 3:6],
                                            in1=epos_t[:, :, 0:3], op=mybir.AluOpType.subtract)
                    nc.vector.tensor_tensor(out=sqt[:], in0=drt[:], in1=drt[:],
                                            op=mybir.AluOpType.mult)
                    nc.vector.tensor_reduce(out=r2t[:], in_=sqt[:],
                                            axis=mybir.AxisListType.X, op=mybir.AluOpType.add)
                    nc.scalar.activation(out=rt[:], in_=r2t[:],
                                         func=mybir.ActivationFunctionType.Sqrt)
                    nc.vector.tensor_scalar(out=r2t[:], in0=rt[:], scalar1=EPS, scalar2=None,
                                            op0=mybir.AluOpType.add)
                    nc.vector.reciprocal(out=rit[:], in_=r2t[:])
                    nc.vector.tensor_tensor(out=dt_[:], in0=drt[:],
                                            in1=rit[:].unsqueeze(2).broadcast_to([128, nt, 3]),
                                            op=mybir.AluOpType.mult)
                    nc.vector.tensor_copy(
                        out=dexp[:],
                        in_=dt_[:].unsqueeze(3).broadcast_to([128, nt, 3, 16]))
                    for k in range(16):
                        nc.scalar.activation(out=bsqt[:, :, k], in_=rt[:],
                                             func=mybir.ActivationFunctionType.Square,
                                             bias=negc[:, k:k + 1])
                    nc.scalar.activation(out=feat_e[:, :, 0:16], in_=bsqt[:],
                                         func=mybir.ActivationFunctionType.Exp,
                                         scale=float(neg_beta))
                    bview = feat_e[:, :, 0:16].unsqueeze(2).broadcast_to([128, nt, 3, 16])
                    nc.vector.tensor_tensor(
                        out=feat_e[:, :, 16:64].rearrange("p t (c k) -> p t c k", c=3),
                        in0=bview, in1=dexp[:], op=mybir.AluOpType.mult)

                    mfeat = msc.tile([128, nblk, 64], f32, tag="mfeat")
                    for b in range(nblk):
                        psA = psA_p.tile([128, 64], f32, tag="psA")
                        for wv in range(4):
                            g = wv
                            for k in range(kt):
                                ti = (b * 4 + wv) * kt + k
                                nc.tensor.matmul(
                                    out=psA[32 * g:32 * g + 32, :],
                                    lhsT=tmat_t[:, ti, :], rhs=feat_e[:, ti, :],
                                    start=(k == 0), stop=(k == kt - 1),
                                    tile_position=(0, 32 * g), skip_group_check=True)
                        nc.scalar.activation(out=mfeat[:, b, :], in_=psA[:],
                                             func=mybir.ActivationFunctionType.Copy)

                    m1e = msc.tile([128, nblk, 3, 24], f32, tag="m1e")
                    m1view = mfeat[:, :, 16:64].rearrange("p b (c k) -> p b c k", c=3)
                    nc.vector.tensor_copy(out=m1e[:, :, :, 0:16], in_=m1view)
                    nc.vector.tensor_copy(out=m1e[:, :, :, 16:24],
                                          in_=m1view[:, :, :, 0:8])

                    feat_at = msc.tile([128, nblk, NFEAT], f32r, tag="feat_at")
                    nc.vector.tensor_copy(out=feat_at[:, :, 0:16], in_=mfeat[:, :, 0:16])
                    g1a = msc.tile([128, nblk, 16, 9], f32, tag="g1a")
                    g1b = msc.tile([128, nblk, 16, 9], f32, tag="g1b")
                    m1v = m1e[:]

                    # pass d: in0[k1, j] = m1e[d, k1], in1[k1, j] = m1e[d, k1 + j]
                    for d, dst in ((0, g1a), (1, g1b)):
                        nc.vector.tensor_tensor(
                            out=dst[:],
                            in0=m1v[:, :, d, 0:16].unsqueeze(3).broadcast_to([128, nblk, 16, 9]),
                            in1=_diag_view(m1v, d, nblk),
                            op=mybir.AluOpType.mult)
                    nc.vector.tensor_tensor(out=g1a[:], in0=g1a[:], in1=g1b[:],
                                            op=mybir.AluOpType.add)
                    nc.vector.tensor_tensor(
                        out=g1b[:],
                        in0=m1v[:, :, 2, 0:16].unsqueeze(3).broadcast_to([128, nblk, 16, 9]),
                        in1=_diag_view(m1v, 2, nblk),
                        op=mybir.AluOpType.mult)
                    nc.vector.tensor_tensor(
                        out=feat_at[:, :, 16:160].rearrange("p b (k j) -> p b k j", k=16),
                        in0=g1a[:], in1=g1b[:], op=mybir.AluOpType.add)

                    fst0 = msc.tile([128, nblk, 128], f32r, tag="fst0")
                    fst1 = msc.tile([128, nblk, 128], f32r, tag="fst1")
                    for b in range(nblk):
                        psT0 = psT_p.tile([128, 128], f32r, tag="psT0")
                        psT1 = psT_p.tile([128, 128], f32r, tag="psT1")
                        nc.tensor.matmul(out=psT0[:], lhsT=feat_at[:, b, 0:128],
                                         rhs=identr, is_transpose=True,
                                         start=True, stop=True, skip_group_check=True)
                        nc.tensor.matmul(out=psT1[:], lhsT=feat_at[:, b, 32:160],
                                         rhs=identr, is_transpose=True,
                                         start=True, stop=True, skip_group_check=True)
                        nc.scalar.activation(out=fst0[:, b, :], in_=psT0[:],
                                             func=mybir.ActivationFunctionType.Copy)
                        nc.vector.tensor_copy(out=fst1[96:128, b, :], in_=psT1[96:128, :])
                    nc.sync.dma_start(out=ft0_d.ap()[:, a0:a0 + nblk * 128],
                                      in_=fst0[:].rearrange("p b x -> p (b x)"))
                    nc.sync.dma_start(out=ft1_d.ap()[:, a0:a0 + nblk * 128],
                                      in_=fst1[96:128].rearrange("p b x -> p (b x)"))
                    blk0 += nblk

            tc.strict_bb_all_engine_barrier()

            # ---------------- Phase B: MLP + readout
            with tc.tile_pool(name="bio", bufs=3) as bio, \
                 tc.tile_pool(name="hmid", bufs=2) as hmid, \
                 tc.tile_pool(name="psB", bufs=3, space="PSUM") as psB_p, \
                 tc.tile_pool(name="psE", bufs=2, space="PSUM") as psE_p:
                for ch in range(N_CH):
                    a0 = ch * ROW_CH
                    w = min(ROW_CH, APC_PAD - a0)
                    f0c = bio.tile([128, w], f32r, tag="f0c")
                    f1c = bio.tile([32, w], f32r, tag="f1c")
                    nc.sync.dma_start(out=f0c[:], in_=ft0_d.ap()[:, a0:a0 + w])
                    nc.sync.dma_start(out=f1c[:], in_=ft1_d.ap()[:, a0:a0 + w])
                    h1s = hmid.tile([128, 4, w], f32r, tag="h1s")
                    for h in range(4):
                        ps = psB_p.tile([128, w], f32, tag="psB")
                        nc.tensor.matmul(out=ps[:], lhsT=w1a_t[:, h * 128:(h + 1) * 128],
                                         rhs=f0c[:], start=True, stop=False, skip_group_check=True)
                        nc.tensor.matmul(out=ps[:], lhsT=w1b_t[:, h * 128:(h + 1) * 128],
                                         rhs=f1c[:], start=False, stop=True, skip_group_check=True)
                        nc.scalar.activation(out=h1s[:, h, :], in_=ps[:],
                                             func=mybir.ActivationFunctionType.Silu,
                                             bias=b1t_t[:, h:h + 1])
                    h2s = hmid.tile([128, 4, w], f32r, tag="h2s")
                    for h in range(4):
                        ps = psB_p.tile([128, w], f32, tag="psB")
                        for k in range(4):
                            nc.tensor.matmul(
                                out=ps[:], lhsT=w2r_t[:, k * HIDDEN + h * 128:k * HIDDEN + (h + 1) * 128],
                                rhs=h1s[:, k, :], start=(k == 0), stop=(k == 3), skip_group_check=True)
                        nc.scalar.activation(out=h2s[:, h, :], in_=ps[:],
                                             func=mybir.ActivationFunctionType.Silu,
                                             bias=b2t_t[:, h:h + 1])
                    pse = psE_p.tile([1, w], f32, tag="psE")
                    for k in range(4):
                        nc.tensor.matmul(out=pse[:], lhsT=w3r_t[:, k:k + 1],
                                         rhs=h2s[:, k, :], start=(k == 0), stop=(k == 3), skip_group_check=True)
                    erow = bio.tile([1, w], f32, tag="erow")
                    nc.scalar.activation(out=erow[:], in_=pse[:],
                                         func=mybir.ActivationFunctionType.Identity,
                                         bias=b3s_t[:])
                    nc.sync.dma_start(out=emat_t[ch:ch + 1, 0:w], in_=erow[:])

                # final readout
                u = bio.tile([EMAT_P, ROW_CH], f32, tag="u")
                acc = bio.tile([EMAT_P, 1], f32, tag="acc")
                nc.vector.tensor_tensor(out=u[:], in0=emat_t[:], in1=smat_t[:],
                                        op=mybir.AluOpType.mult)
                nc.vector.scalar_tensor_tensor(out=u[:], in0=u[:], scalar=1.0, in1=shm_t[:],
                                               op0=mybir.AluOpType.mult,
                                               op1=mybir.AluOpType.add, accum_out=acc[:])
                ones = bio.tile([EMAT_P, 1], f32, tag="ones")
                nc.vector.memset(ones[:], 1.0)
                psf = psE_p.tile([1, 1], f32, tag="psf")
                nc.tensor.matmul(out=psf[:], lhsT=ones[:], rhs=acc[:], start=True, stop=True, skip_group_check=True)
                eo = bio.tile([1, 1], f32, tag="eo")
                nc.scalar.activation(out=eo[:], in_=psf[:],
                                     func=mybir.ActivationFunctionType.Copy)
                nc.sync.dma_start(out=eout_d.ap(), in_=eo[:])

    _PROG_CACHE[key] = nc
    return nc


def _diag_view(m1v, d, nblk):
    """AP [128, nblk, k1(16), j(9)] reading m1e[:, :, d, k1 + j] (overlapping
    windows: both trailing dims step 1)."""
    import concourse.ap as cap

    base = m1v[:, :, d, :]                         # [128, nblk, 24]
    v = base.unsqueeze(3)                          # [128, nblk, 24, 1]
    v = v[:, :, 0:16, :]                           # [128, nblk, 16, 1]
    v = v.broadcast_to([128, nblk, 16, 9])         # last dim stride 0
    apl = [list(p) for p in v.ap]
    apl[-1] = [1, 9]
    return cap.AP(v.tensor, v.offset, apl, v.const_val, v.runtime_checks,
                  v.dep_tracking_offset)


LAST_EXEC_NS = None
PROFILE = False


def kernel(**inputs):
    from concourse.bass_utils import run_bass_kernel_spmd

    consts, per_core, shared = _preprocess(**inputs)
    nc = _get_program(consts["kt"], consts["centers"], consts["neg_beta"],
                      float(np.asarray(shared["b3s"]).reshape(())))
    in_maps = []
    for c in range(N_CORES):
        m = dict(
            epos=per_core["epos"][c],
            tmat=per_core["tmat"][c],
            w1a=shared["w1a"], w1b=shared["w1b"],
            w2r=shared["w2r"], w3r=shared["w3r"],
            b1t=shared["b1t"], b2t=shared["b2t"], b3s=shared["b3s"],
            smat=per_core["smat"][c], shm=per_core["shmat"][c],
        )
        in_maps.append(m)
    global LAST_EXEC_NS
    kwargs = {}
    if PROFILE:
        import tempfile
        kwargs = dict(trace=True, tmpdir=tempfile.mkdtemp(prefix="ktrace_"))
    res = run_bass_kernel_spmd(nc, in_maps, core_ids=list(range(N_CORES)), **kwargs)
    if getattr(res, "exec_time_ns", None):
        LAST_EXEC_NS = res.exec_time_ns
    if PROFILE:
        globals()["LAST_RESULTS"] = res
    total = np.float32(0.0)
    for c in range(N_CORES):
        total += np.float32(res.results[c]["eout"].reshape(()))
    return np.float32(total)

